# revision 17
# baseline (speedup 1.0000x reference)
"""Additive (Bahdanau) cross-attention kernel for 8 TRN2 NeuronCores.

Math: scores[b,q,k] = sum_h v[h] * tanh(qh[b,q,h] + kh[b,k,h])
      weights = softmax_k(scores); out = weights @ values

Key trick: tanh(z) ~= sum_j b_j * sin(w_j z) (Fourier sine series, max err
2.7e-3 on |z|<=5), and sin(w(qh+kh)) = sin(w qh)cos(w kh) + cos(w qh)sin(w kh)
separates per-(q,k) work into rank-H matmuls: the O(LQ*LK*H) tanh evaluations
become 2J TensorEngine matmuls plus O((LQ+LK)*H) sin/cos ACT-engine evals.

Sharding: batch (4) x query-half (2) -> 8 cores, keys/values replicated per
batch pair; no collectives.

Hardware quirk honored throughout: PE transpose (S3_LW) instructions carry at
most ONE semaphore wait, so every transpose's inputs (source tile, identity,
PSUM slot) must depend on a single engine -- all transpose sources are
DVE-produced bf16 tiles and transpose PSUM slots are freed by DVE copies.
"""

import numpy as np
from contextlib import ExitStack

import concourse.bass as bass
import concourse.mybir as mybir
import concourse.tile as tile
from concourse.bass_utils import run_bass_kernel_spmd
from concourse.masks import make_identity

B, LQ, LK, D, H = 4, 256, 1024, 512, 128
QS = LQ // 2      # 128 queries per core
NCORE = 8
DCH = D // 128    # 4 contraction chunks
KT = LK // 128    # 8 key tiles

# tanh(z) ~= sum_j BCOEF[j] * sin(GRID[j]*OMEGA1*z); maxerr 2.7e-2, rms@data
# 1.6e-3.  Only GRID 1,2,3 are evaluated by the ACT Sin table (args stay
# within its accurate |x|<~pi window); 4, 6, 8 come from exact double-angle
# products.  GAMMA[j] compensates the 1/2-per-doubling in the product tiles.
OMEGA1 = 0.4487989505128276
GRID = [1, 2, 3, 4, 6, 8]
BCOEF = [1.1499596596, 0.0461030978, 0.1584585002, 0.0631930252,
         0.033470942, 0.0043143511]
GAMMA = {1: 1.0, 2: 1.0, 3: 1.0, 4: 2.0, 6: 2.0, 8: 4.0}
DERIVED = {4: 2, 6: 3, 8: 4}   # freq -> source freq (doubling)
J = len(GRID)
HALF_PI = 1.5707963267948966

f32 = mybir.dt.float32
bf16 = mybir.dt.bfloat16

_CACHE = {}


def _build():
    nc = bass.Bass("TRN2")
    # Register pi/2 as an init-time const AP (like bass's built-in 0.0/1.0) so
    # activation(bias=HALF_PI) carries no runtime dependency -- instructions
    # here can hold at most one cross-engine semaphore wait.
    _hp = nc.alloc_sbuf_tensor("const-f32-halfpi", [128, 1], f32)
    nc.gpsimd.memset(_hp.ap(), HALF_PI)
    nc.const_aps.aps[(f32, HALF_PI)] = _hp.ap()
    nc.all_engine_barrier()
    # qw = [query_shard | Wq | Wk | v] packed host-side: one DMA, one sem lane
    d_qw = nc.dram_tensor("qw", [128, 3 * D + 1], f32, kind="ExternalInput")
    d_keys = nc.dram_tensor("keys", [LK, D], f32, kind="ExternalInput")
    d_vals = nc.dram_tensor("values", [LK, D], f32, kind="ExternalInput")
    # outs = [weights (LK) | out (D)] packed: one store DMA
    d_outs = nc.dram_tensor("outs", [QS, LK + D], f32, kind="ExternalOutput")

    Sin = mybir.ActivationFunctionType.Sin
    Exp = mybir.ActivationFunctionType.Exp
    Copy = mybir.ActivationFunctionType.Copy
    mult = mybir.AluOpType.mult
    add = mybir.AluOpType.add

    with tile.TileContext(nc) as tc, ExitStack() as ctx:
        const = ctx.enter_context(tc.tile_pool(name="const", bufs=1))
        ldp = ctx.enter_context(tc.tile_pool(name="ldp", bufs=2))
        persist = ctx.enter_context(tc.tile_pool(name="persist", bufs=1))
        harm_k = ctx.enter_context(tc.tile_pool(name="harm_k", bufs=1))
        harm_q = ctx.enter_context(tc.tile_pool(name="harm_q", bufs=1))
        tailp = ctx.enter_context(tc.tile_pool(name="tailp", bufs=1))
        ps_tr = ctx.enter_context(tc.tile_pool(name="ps_tr", bufs=2, space="PSUM"))
        ps_qh = ctx.enter_context(tc.tile_pool(name="ps_qh", bufs=1, space="PSUM"))
        ps_kh = ctx.enter_context(tc.tile_pool(name="ps_kh", bufs=1, space="PSUM"))
        ps_sc = ctx.enter_context(tc.tile_pool(name="ps_sc", bufs=2, space="PSUM"))
        ps_out = ctx.enter_context(tc.tile_pool(name="ps_out", bufs=1, space="PSUM"))

        # identity for PE transposes: DVE-stamped so transposes only wait on DVE
        id_gp = const.tile([128, 128], bf16, tag="id_gp")
        make_identity(nc, id_gp[:])
        id_bf = const.tile([128, 128], bf16, tag="id_bf")
        nc.vector.tensor_copy(id_bf[:], id_gp[:])

        qw_sb = const.tile([128, 3 * D + 1], f32, tag="qw_sb")
        nc.sync.dma_start(out=qw_sb[:], in_=d_qw[:])
        # ACT-owned copy: the DVE folds' two deps (sinq from ACT, v_sb) then
        # merge onto the single ACT semaphore
        v_sb = const.tile([128, 1], f32, tag="v_sb")
        nc.scalar.copy(v_sb[:], qw_sb[:, 3 * D:3 * D + 1])

        def transpose_group(pairs):
            """Transpose up to 4 [128,128] blocks through one [128,512] bf16
            PSUM tile, freed by a single DVE copy. pairs = [(dst_ap, src_ap)].
            Sources must be DVE-produced bf16 (single-wait discipline)."""
            p = ps_tr.tile([128, 512], bf16, tag="tr", name="tr_p")
            for i, (_, src_ap) in enumerate(pairs):
                nc.tensor.transpose(p[:, i * 128:(i + 1) * 128], src_ap, id_bf[:])
            for i, (dst_ap, _) in enumerate(pairs):
                nc.vector.tensor_copy(dst_ap, p[:, i * 128:(i + 1) * 128])

        # ---- cast + transpose query / Wq / Wk -> [d, .] bf16 ----
        def load_transposed(idx, tag):
            src_bf = ldp.tile([128, D], bf16, tag=f"ldbf_{tag}", name=f"ldbf_{tag}")
            nc.vector.tensor_copy(src_bf[:], qw_sb[:, idx * D:(idx + 1) * D])
            dstT = persist.tile([128, DCH * 128], bf16, tag=tag, name=tag)
            transpose_group([(dstT[:, c * 128:(c + 1) * 128],
                              src_bf[:, c * 128:(c + 1) * 128])
                             for c in range(DCH)])
            return dstT

        queryT = load_transposed(0, "queryT")
        WqT = load_transposed(1, "WqT")
        WkT = load_transposed(2, "WkT")

        # ---- keys: 2x1MB loads, cast, transpose to keysT[c] = [d_chunk, LK] ----
        keysT = [persist.tile([128, LK], bf16, tag=f"keysT{c}", name=f"keysT{c}")
                 for c in range(DCH)]
        kf = ldp.tile([128, KT, D], f32, tag="keysf", name="keysf")
        nc.sync.dma_start(out=kf[:], in_=d_keys[:].rearrange("(t p) d -> p t d", p=128))
        kf_bf = ldp.tile([128, KT, D], bf16, tag="keysf_bf", name="keysf_bf")
        nc.vector.tensor_copy(kf_bf[:], kf[:])
        for kt in range(KT):
            transpose_group([(keysT[c][:, kt * 128:(kt + 1) * 128],
                              kf_bf[:, kt, c * 128:(c + 1) * 128])
                             for c in range(DCH)])

        # ---- projections: qhT [h, q], khT [h, k] (fp32 PSUM) ----
        qhT = ps_qh.tile([128, 128], f32, tag="qhT")
        for c in range(DCH):
            nc.tensor.matmul(qhT[:], WqT[:, c * 128:(c + 1) * 128],
                             queryT[:, c * 128:(c + 1) * 128],
                             start=(c == 0), stop=(c == DCH - 1))
        khT = ps_kh.tile([128, LK], f32, tag="khT")
        for kh in range(2):
            sl = slice(kh * 512, (kh + 1) * 512)
            for c in range(DCH):
                nc.tensor.matmul(khT[:, sl], WkT[:, c * 128:(c + 1) * 128],
                                 keysT[c][:, sl],
                                 start=(c == 0), stop=(c == DCH - 1))

        # ---- values: 2x1MB loads on the ACT HWDGE ring + gpsimd casts ----
        vals_bf = []
        vf = ldp.tile([128, KT, D], f32, tag="valsf", name="valsf")
        nc.scalar.dma_start(out=vf[:],
                            in_=d_vals[:].rearrange("(t p) d -> p t d", p=128))
        for t in range(KT):
            vb = persist.tile([128, D], bf16, tag=f"vals_bf{t}",
                              name=f"vals_bf{t}")
            nc.vector.tensor_copy(vb[:], vf[:, t, :])
            vals_bf.append(vb)

        # ---- harmonics + score matmuls ----
        scores = [ps_sc.tile([128, 512], f32, tag="scores", name=f"scores{i}")
                  for i in range(2)]
        sub = mybir.AluOpType.subtract
        kt_s, kt_c, qt_s, qt_c = {}, {}, {}, {}
        for jf in (1, 2, 3):
            w = jf * OMEGA1
            s = harm_k.tile([128, LK], bf16, tag=f"sink{jf}", name=f"sink{jf}")
            nc.scalar.activation(s[:], khT[:], Sin, bias=0.0, scale=w)
            c = harm_k.tile([128, LK], bf16, tag=f"cosk{jf}", name=f"cosk{jf}")
            nc.scalar.activation(c[:], khT[:], Sin, bias=HALF_PI, scale=w)
            kt_s[jf], kt_c[jf] = s, c
            s = harm_q.tile([128, 128], bf16, tag=f"sinq{jf}", name=f"sinq{jf}")
            nc.scalar.activation(s[:], qhT[:], Sin, bias=0.0, scale=w)
            c = harm_q.tile([128, 128], bf16, tag=f"cosq{jf}", name=f"cosq{jf}")
            nc.scalar.activation(c[:], qhT[:], Sin, bias=HALF_PI, scale=w)
            qt_s[jf], qt_c[jf] = s, c
        for jf, sf in DERIVED.items():
            # sin-tile carries a 1/2 (folded via GAMMA): s~_2f = s_f * c_f
            # cos is exact: c_2f = 1 - 2*gamma_f^2 * s~_f^2
            g2 = -2.0 * GAMMA[sf] * GAMMA[sf]
            for (st, ct, shape, pool) in ((kt_s, kt_c, LK, harm_k),
                                          (qt_s, qt_c, 128, harm_q)):
                s = pool.tile([128, shape], bf16, tag=f"sd{jf}_{shape}",
                              name=f"sd{jf}_{shape}")
                nc.vector.tensor_tensor(s[:], st[sf][:], ct[sf][:], mult)
                c = pool.tile([128, shape], bf16, tag=f"cd{jf}_{shape}",
                              name=f"cd{jf}_{shape}")
                nc.vector.tensor_tensor(c[:], st[sf][:], st[sf][:], mult)
                nc.vector.tensor_scalar(c[:], c[:], float(g2), 1.0, mult, add)
                st[jf], ct[jf] = s, c
        for j, jf in enumerate(GRID):
            bg = float(BCOEF[j] * GAMMA[jf])
            lhs_s = harm_q.tile([128, 128], bf16, tag=f"lhs_s{jf}",
                                name=f"lhs_s{jf}")
            nc.vector.tensor_scalar(lhs_s[:], qt_s[jf][:], v_sb[:], bg, mult, mult)
            lhs_c = harm_q.tile([128, 128], bf16, tag=f"lhs_c{jf}",
                                name=f"lhs_c{jf}")
            nc.vector.tensor_scalar(lhs_c[:], qt_c[jf][:], v_sb[:], bg, mult, mult)
            for kh in range(2):
                sl = slice(kh * 512, (kh + 1) * 512)
                nc.tensor.matmul(scores[kh][:], lhs_s[:], kt_c[jf][:, sl],
                                 start=(j == 0), stop=False)
                nc.tensor.matmul(scores[kh][:], lhs_c[:], kt_s[jf][:, sl],
                                 start=False, stop=(j == J - 1))

        # ---- softmax (scores are O(1): no max-subtraction needed) ----
        exp_f, sums = [], []
        for kh in range(2):
            e = tailp.tile([128, 512], f32, tag=f"exp_f{kh}", name=f"exp_f{kh}")
            s = tailp.tile([128, 1], f32, tag=f"sum{kh}", name=f"sum{kh}")
            nc.scalar.activation(e[:], scores[kh][:], Exp, bias=0.0, scale=1.0,
                                 accum_out=s[:])
            exp_f.append(e)
            sums.append(s)
        sumtot = tailp.tile([128, 1], f32, tag="sumtot")
        nc.vector.tensor_tensor(sumtot[:], sums[0][:], sums[1][:], add)
        recip = tailp.tile([128, 1], f32, tag="recip")
        nc.vector.reciprocal(recip[:], sumtot[:])

        # bf16 copies of exp first: these sync DVE to ACT, so the later
        # normalizes only carry their DVE self-drain wait
        exp_bf = []
        for kh in range(2):
            eb = tailp.tile([128, 512], bf16, tag=f"exp_bf{kh}", name=f"exp_bf{kh}")
            nc.vector.tensor_copy(eb[:], exp_f[kh][:])
            exp_bf.append(eb)

        # weights output (fp32, normalized) into the packed output tile
        outs_sb = tailp.tile([128, LK + D], f32, tag="outs_sb", name="outs_sb")
        for kh in range(2):
            nc.vector.tensor_scalar(outs_sb[:, kh * 512:(kh + 1) * 512],
                                    exp_f[kh][:], recip[:], None, mult)
        outp = ps_out.tile([128, D], f32, tag="outp")
        wTs = []
        for g in range(2):
            wT = tailp.tile([128, 512], bf16, tag=f"wT{g}", name=f"wT{g}")
            transpose_group([(wT[:, i * 128:(i + 1) * 128],
                              exp_bf[g][:, i * 128:(i + 1) * 128])
                             for i in range(4)])
            wTs.append(wT)
        for t in range(KT):
            nc.tensor.matmul(outp[:], wTs[t // 4][:, (t % 4) * 128:(t % 4 + 1) * 128],
                             vals_bf[t][:], start=(t == 0), stop=(t == KT - 1))
        nc.vector.tensor_scalar(outs_sb[:, LK:], outp[:], recip[:], None, mult)
        nc.sync.dma_start(out=d_outs[:], in_=outs_sb[:])

    return nc



def _wait_limit(inst):
    op = inst.get("opcode")
    if op == "Matmult":
        return 1 if inst.get("is_transpose") else 2
    return 1


def _split_excess_waits(raw):
    """Walrus enforces tiny per-instruction sync-wait budgets (1 for most ops,
    2 for Drain/regular Matmult). Tile sometimes emits more (notably the
    kernel-tail drain, which waits on every engine + DMA lane). Hoist the
    excess into preceding same-engine Drain instructions."""
    import json as _json
    d = _json.loads(raw)
    n_split = 0
    for fn in d.get("functions", []):
        for bb in fn.get("blocks", []):
            insts = bb.get("instructions", [])
            out = []
            for inst in insts:
                si = inst.get("sync_info") or {}
                waits = si.get("on_wait") or []
                lim = _wait_limit(inst)
                if len(waits) > lim:
                    excess, keep = waits[:-lim], waits[-lim:]
                    for i, wcmd in enumerate(excess):
                        n_split += 1
                        out.append({
                            "debug": inst.get("debug"),
                            "engine": inst["engine"],
                            "ins": [], "outs": [],
                            "name": f"{inst['name']}-ws{i}",
                            "opcode": "Drain",
                            "sync_info": {"on_wait": [wcmd]},
                        })
                    si["on_wait"] = keep
                    inst["sync_info"] = si
                out.append(inst)
            bb["instructions"] = out
    return _json.dumps(d).encode()


def _patch_json(nc):
    orig = nc.to_json_bytes

    def patched():
        return _split_excess_waits(orig())

    nc.to_json_bytes = patched


def _get_nc():
    if "nc" not in _CACHE:
        nc = _build()
        _patch_json(nc)
        _CACHE["nc"] = nc
    return _CACHE["nc"]


def _run(inputs, trace=False):
    nc = _get_nc()
    query = np.asarray(inputs["query"], dtype=np.float32)
    keys = np.asarray(inputs["keys"], dtype=np.float32)
    values = np.asarray(inputs["values"], dtype=np.float32)
    Wq = np.ascontiguousarray(np.asarray(inputs["Wq"], dtype=np.float32))
    Wk = np.ascontiguousarray(np.asarray(inputs["Wk"], dtype=np.float32))
    v = np.asarray(inputs["v"], dtype=np.float32)

    in_maps = []
    for c in range(NCORE):
        b, qh = c // 2, c % 2
        qw = np.concatenate(
            [query[b, qh * QS:(qh + 1) * QS, :], Wq, Wk, v.reshape(H, 1)], axis=1)
        in_maps.append({
            "qw": np.ascontiguousarray(qw),
            "keys": np.ascontiguousarray(keys[b]),
            "values": np.ascontiguousarray(values[b]),
        })
    res = run_bass_kernel_spmd(nc, in_maps, core_ids=list(range(NCORE)),
                               trace=trace)
    out = np.zeros((B, LQ, D), dtype=np.float32)
    wout = np.zeros((B, LQ, LK), dtype=np.float32)
    for c in range(NCORE):
        b, qh = c // 2, c % 2
        outs = res.results[c]["outs"]
        wout[b, qh * QS:(qh + 1) * QS, :] = outs[:, :LK]
        out[b, qh * QS:(qh + 1) * QS, :] = outs[:, LK:]
    return (out, wout), res


def kernel(query, keys, values, Wq, Wk, v):
    (out, wout), _ = _run(dict(query=query, keys=keys, values=values,
                               Wq=Wq, Wk=Wk, v=v))
    return (out, wout)
